# revision 3
# baseline (speedup 1.0000x reference)
"""Conv2D (VALID, 3x3, NCHW) on 8 TRN2 NeuronCores via Bass/Tile.

Problem: x (32,128,56,56) f32, weight (256,128,3,3) f32, bias (256,) f32
         -> out (32,256,54,54) f32.

Strategy:
  - Data-parallel over batch: 4 images per core, 8 cores, no collectives.
  - Conv as implicit GEMM: for each kernel tap (kh,kw), a matmul with
    lhsT = weight[ci, co_tile] (K=Cin=128 partitions, M=128) and
    rhs  = shifted x window [Cin=128, 9 rows x 54 cols = 486], accumulating
    all 9 taps into one PSUM bank. 2 cout tiles x 6 row groups x 4 images
    = 48 accumulation groups x 9 matmuls per core.
  - Inputs cast to bf16 on host (PE runs at full rate); accumulation fp32.
  - Bias added during the PSUM->SBUF copy on DVE, then DMA out as f32.
"""

import numpy as np
import ml_dtypes

import concourse.bass as bass
import concourse.mybir as mybir
from concourse import bacc
import concourse.tile as tile
from concourse.bass_utils import run_bass_kernel_spmd

N, CIN, H, W = 32, 128, 56, 56
COUT, KH, KW = 256, 3, 3
HO, WO = H - KH + 1, W - KW + 1  # 54, 54
NCORES = 8
NPER = N // NCORES  # 4 images per core
CTILES = COUT // 128  # 2
RG = 9                # output rows per PSUM group
NG = HO // RG         # 6 row groups
NPIX = RG * WO        # 486 <= 512 (one fp32 PSUM bank)

BF16 = mybir.dt.bfloat16
F32 = mybir.dt.float32


def build_nc() -> bass.Bass:
    nc = bacc.Bacc(None)
    x_h = nc.dram_tensor("x", [NPER, CIN, H, W], BF16, kind="ExternalInput")
    w_h = nc.dram_tensor("w", [CIN, KH * KW * COUT], BF16, kind="ExternalInput")
    b_h = nc.dram_tensor("b", [COUT, 1], F32, kind="ExternalInput")
    o_h = nc.dram_tensor("out", [NPER, COUT, HO, WO], F32, kind="ExternalOutput")

    with tile.TileContext(nc) as tc:
        with (
            tc.tile_pool(name="wpool", bufs=1) as wpool,
            tc.tile_pool(name="bpool", bufs=1) as bpool,
            tc.tile_pool(name="xpool", bufs=3) as xpool,
            tc.tile_pool(name="opool", bufs=4) as opool,
            tc.tile_pool(name="psum", bufs=8, space="PSUM") as psum_pool,
        ):
            wt = wpool.tile([CIN, KH * KW * COUT], BF16)
            nc.sync.dma_start(out=wt[:], in_=w_h[:])
            bias_t = bpool.tile([COUT // CTILES, CTILES], F32)
            for c in range(CTILES):
                nc.sync.dma_start(out=bias_t[:, c : c + 1], in_=b_h[c * 128 : (c + 1) * 128, :])

            for n in range(NPER):
                xt = xpool.tile([CIN, H, W], BF16, tag="xt")
                nc.sync.dma_start(out=xt[:], in_=x_h[n])
                for c in range(CTILES):
                    for g in range(NG):
                        pt = psum_pool.tile([128, RG, WO], F32, tag="pt")
                        for t in range(KH * KW):
                            kh, kw = divmod(t, KW)
                            lhsT = wt[:, t * COUT + c * 128 : t * COUT + c * 128 + 128]
                            rhs = xt[:, g * RG + kh : g * RG + kh + RG, kw : kw + WO]
                            nc.tensor.matmul(
                                pt[:], lhsT, rhs,
                                start=(t == 0), stop=(t == KH * KW - 1),
                            )
                        ot = opool.tile([128, RG, WO], F32, tag="ot")
                        nc.vector.tensor_scalar_add(ot[:], pt[:], bias_t[:, c : c + 1])
                        nc.sync.dma_start(
                            out=o_h[n, c * 128 : (c + 1) * 128, g * RG : (g + 1) * RG, :],
                            in_=ot[:],
                        )
    nc.finalize()
    return nc


_NC_CACHE = None


def _get_nc():
    global _NC_CACHE
    if _NC_CACHE is None:
        _NC_CACHE = build_nc()
    return _NC_CACHE


def _prep_in_maps(x, weight, bias):
    bf16 = ml_dtypes.bfloat16
    # [ci, kh, kw, co] layout so lhsT slices are [ci, co_tile]
    w_t = np.ascontiguousarray(
        weight.astype(np.float32).transpose(1, 2, 3, 0).reshape(CIN, KH * KW * COUT)
    ).astype(bf16)
    b_t = np.ascontiguousarray(bias.astype(np.float32).reshape(COUT, 1))
    in_maps = []
    for i in range(NCORES):
        xs = np.ascontiguousarray(x[i * NPER : (i + 1) * NPER]).astype(bf16)
        in_maps.append({"x": xs, "w": w_t, "b": b_t})
    return in_maps


def run(x, weight, bias, trace=False):
    nc = _get_nc()
    in_maps = _prep_in_maps(x, weight, bias)
    res = run_bass_kernel_spmd(nc, in_maps, core_ids=list(range(NCORES)), trace=trace)
    out = np.concatenate([r["out"] for r in res.results], axis=0)
    return out, res


def kernel(x: np.ndarray, weight: np.ndarray, bias: np.ndarray) -> np.ndarray:
    out, _ = run(x, weight, bias, trace=False)
    return out.astype(np.float32)


if __name__ == "__main__":
    nc = build_nc()
    print("built ok:", len(nc.m.functions[0].blocks if hasattr(nc.m.functions[0], 'blocks') else []), "blocks")
